# revision 1
# baseline (speedup 1.0000x reference)
"""MoE head kernel for Trainium2 (8 NeuronCores, data-parallel over batch).

Computes, per the reference nn.Module:
  w      = softmax(cos_sim(z_cat, mu_cat) / tau)          # gate  [B, E]
  xhat   = LayerNorm(feat)  (no affine applied yet)
  x_e    = xhat * gamma_e + beta_e                         # per-expert affine
  h_e    = relu(x_e @ W1_e + b1_e)
  l_e    = h_e @ W2_e + b2_e
  logits = sum_e w[:, e] * l_e                             # [B, C]
returns (logits, w).

Sharding: batch B=16384 split 8 ways (2048 rows/core); all params replicated.
No collectives. Everything computed on-device; outputs gathered on host.

Layout strategy per core:
  - LN in [B, D] layout (rows on partitions), then PE-transpose to
    xhatT [D, B] so the D-contraction matmul has D on partitions.
  - mm1: out hT [H-tile(128), Bchunk(512)] = W1_strip.T @ xhatT, accumulated
    over 8 K-tiles in PSUM; fused bias+relu on ScalarE into SBUF.
  - mm2: out lT [8, Bchunk] = W2_strip.T @ hT, accumulated over 16 H-tiles
    in PSUM (one bank per B-chunk, 4 chunks live at once).
  - lT + b2 -> PE-transpose back to [B-tile, 8] -> scale by gate column
    w[:, e] (a per-partition scalar in this layout) -> accumulate logits.
Matmul operands are bitcast to float32r (full-rate fp32 on the PE when the
moving free dim >= 256; mm1 rhs is 512 wide).
"""

import numpy as np
from contextlib import ExitStack

import concourse.bass as bass
import concourse.mybir as mybir
import concourse.tile as tile
from concourse import bacc
from concourse.masks import make_identity
from concourse.bass_utils import run_bass_kernel_spmd

# Problem shapes (hardcoded per contract).
B, D, H, E, DZ = 16384, 1024, 2048, 8, 256
NCORES = 8
BS = B // NCORES            # rows per core = 2048
CHUNK = 512                 # batch chunk for matmul free dim
NCH = BS // CHUNK           # 4
BT = BS // 128              # 16 partition tiles of batch
KD = D // 128               # 8 K-tiles for mm1
MH = H // 128               # 16 M-tiles of hidden
KZ = DZ // 128              # 2 K-tiles for the gate matmul
LN_EPS = 1e-5

F32 = mybir.dt.float32
AF = mybir.ActivationFunctionType
ALU = mybir.AluOpType
AX = mybir.AxisListType


def _build(tau: float, affine: bool, mm_dt=mybir.dt.float32r):
    nc = bacc.Bacc(None, target_bir_lowering=False, name="moe_head")

    feat = nc.dram_tensor("feat", [BS, D], F32, kind="ExternalInput")
    z = nc.dram_tensor("z", [BS, DZ], F32, kind="ExternalInput")
    mu = nc.dram_tensor("mu", [E, DZ], F32, kind="ExternalInput")
    w1 = nc.dram_tensor("w1", [E, D, H], mm_dt, kind="ExternalInput")
    b1 = nc.dram_tensor("b1", [E, H], F32, kind="ExternalInput")
    w2 = nc.dram_tensor("w2", [E, H, E], mm_dt, kind="ExternalInput")
    b2 = nc.dram_tensor("b2", [E, E], F32, kind="ExternalInput")
    if affine:
        gam = nc.dram_tensor("gam", [E, D], F32, kind="ExternalInput")
        bet = nc.dram_tensor("bet", [E, D], F32, kind="ExternalInput")
    logits_o = nc.dram_tensor("logits", [BS, E], F32, kind="ExternalOutput")
    w_o = nc.dram_tensor("w", [BS, E], F32, kind="ExternalOutput")

    inv_tau = 1.0 / tau

    with tile.TileContext(nc) as tc, ExitStack() as ctx:
        persist = ctx.enter_context(tc.tile_pool(name="persist", bufs=1))
        lnpool = ctx.enter_context(tc.tile_pool(name="ln", bufs=3))
        statp = ctx.enter_context(tc.tile_pool(name="stat", bufs=4))
        wpool = ctx.enter_context(tc.tile_pool(name="w1s", bufs=3))
        epool = ctx.enter_context(tc.tile_pool(name="eparam", bufs=2))
        hpool = ctx.enter_context(tc.tile_pool(name="h", bufs=6))
        spool = ctx.enter_context(tc.tile_pool(name="small", bufs=3))
        psA = ctx.enter_context(tc.tile_pool(name="psA", bufs=2, space="PSUM"))
        psB = ctx.enter_context(tc.tile_pool(name="psB", bufs=4, space="PSUM"))
        psC = ctx.enter_context(tc.tile_pool(name="psC", bufs=2, space="PSUM"))

        # Persistent SBUF tensors.
        # xhatT split per B-chunk so the expert loop can start on chunk 0
        # while LN/transpose still runs on later chunks.
        xhatT_c = [persist.tile([128, KD, CHUNK], mm_dt, name=f"xhatT{c}")
                   for c in range(NCH)]
        znT = persist.tile([128, KZ, BS], F32)        # normalized z, transposed
        munT = persist.tile([128, KZ, E], F32)        # normalized mu, transposed
        w_sb = persist.tile([128, BT, E], F32)        # gate weights [B, E]
        acc = persist.tile([128, BT, E], F32)         # logits accumulator [B, C]
        ident = persist.tile([128, 128], F32)
        # b2 columns replicated at partition groups 0/32/64/96 — one copy per
        # mm2 col-group band (band j = B-chunk j's expert logits).
        b2T4 = persist.tile([128, E], F32)
        eps_sb = persist.tile([128, 1], F32)
        if affine:
            gamT = persist.tile([128, KD, E], F32)
            betT = persist.tile([128, KD, E], F32)
            x_eT = persist.tile([128, KD, BS], mm_dt)  # per-expert affine input

        make_identity(nc, ident)
        nc.vector.memset(acc[:], 0.0)
        nc.vector.memset(eps_sb[:], LN_EPS)
        with nc.allow_non_contiguous_dma(reason="tiny strided param loads"):
            for j in range(NCH):
                nc.sync.dma_start(
                    b2T4[32 * j:32 * j + E, :], b2.rearrange("e c -> c e"))
            if affine:
                nc.sync.dma_start(
                    gamT[:], gam.rearrange("e (ko ki) -> ki ko e", ki=128))
                nc.sync.dma_start(
                    betT[:], bet.rearrange("e (ko ki) -> ki ko e", ki=128))

        # ---------------- Phase 0a: gate ----------------
        # mu: normalize rows of [E, DZ], transpose to munT.
        mu_sb = spool.tile([E, DZ], F32, tag="mu")
        nc.sync.dma_start(mu_sb[:], mu[:, :])
        musq = spool.tile([E, DZ], F32, tag="musq")
        muss = statp.tile([E, 1], F32, tag="muss")
        nc.scalar.activation(musq, mu_sb, AF.Square, accum_out=muss)
        mustd = statp.tile([E, 1], F32, tag="mustd")
        nc.scalar.activation(mustd, muss, AF.Sqrt)
        murn = statp.tile([E, 1], F32, tag="murn")
        nc.vector.reciprocal(murn, mustd)
        mu_n = spool.tile([E, DZ], F32, tag="mun")
        nc.vector.tensor_scalar_mul(mu_n[:], mu_sb[:], murn)
        for kz in range(KZ):
            pst = psC.tile([128, 128], F32, tag="tp")
            nc.tensor.transpose(
                pst[:, :E], mu_n[:, kz * 128:(kz + 1) * 128], ident[:E, :E])
            nc.vector.tensor_copy(munT[:, kz, :], pst[:, :E])

        # z: normalize rows tile-by-tile, transpose into znT.
        for bt in range(BT):
            bsl = slice(bt * 128, (bt + 1) * 128)
            zt = lnpool.tile([128, DZ], F32, tag="zt")
            nc.sync.dma_start(zt[:], z[bsl, :])
            zsq = lnpool.tile([128, DZ], F32, tag="zsq")
            zss = statp.tile([128, 1], F32, tag="zss")
            nc.scalar.activation(zsq, zt, AF.Square, accum_out=zss)
            zstd = statp.tile([128, 1], F32, tag="zstd")
            nc.scalar.activation(zstd, zss, AF.Sqrt)
            zrn = statp.tile([128, 1], F32, tag="zrn")
            nc.vector.reciprocal(zrn, zstd)
            zn = lnpool.tile([128, DZ], F32, tag="zn")
            nc.vector.tensor_scalar_mul(zn[:], zt[:], zrn)
            for kz in range(KZ):
                pst = psC.tile([128, 128], F32, tag="tp")
                nc.tensor.transpose(
                    pst[:], zn[:, kz * 128:(kz + 1) * 128], ident[:])
                nc.vector.tensor_copy(znT[:, kz, bsl], pst[:])

        # sims + softmax per batch tile -> w_sb.
        for bt in range(BT):
            bsl = slice(bt * 128, (bt + 1) * 128)
            ps = psC.tile([128, E], F32, tag="tp")
            for kz in range(KZ):
                nc.tensor.matmul(
                    ps[:], znT[:, kz, bsl], munT[:, kz, :],
                    start=(kz == 0), stop=(kz == KZ - 1))
            mx = statp.tile([128, 1], F32, tag="mx")
            nc.vector.reduce_max(mx, ps[:], axis=AX.X)
            nb = statp.tile([128, 1], F32, tag="nb")
            nc.vector.tensor_scalar_mul(nb, mx, -inv_tau)
            ex = spool.tile([128, E], F32, tag="ex")
            nc.scalar.activation(ex[:], ps[:], AF.Exp, bias=nb, scale=inv_tau)
            sm = statp.tile([128, 1], F32, tag="sm")
            nc.vector.reduce_sum(sm, ex[:], axis=AX.X)
            rsm = statp.tile([128, 1], F32, tag="rsm")
            nc.vector.reciprocal(rsm, sm)
            nc.vector.tensor_scalar_mul(w_sb[:, bt, :], ex[:], rsm)

        # ---------------- Phase 0b: LayerNorm + transpose ----------------
        for bt in range(BT):
            bsl = slice(bt * 128, (bt + 1) * 128)
            ft = lnpool.tile([128, D], F32, tag="ft")
            nc.sync.dma_start(ft[:], feat[bsl, :])
            s1 = statp.tile([128, 1], F32, tag="s1")
            nc.vector.reduce_sum(s1, ft[:], axis=AX.X)
            nm = statp.tile([128, 1], F32, tag="nm")
            nc.vector.tensor_scalar_mul(nm, s1, -1.0 / D)
            xc = lnpool.tile([128, D], F32, tag="xc")
            nc.vector.tensor_scalar_add(xc[:], ft[:], nm)
            sq = lnpool.tile([128, D], F32, tag="sq")
            ss = statp.tile([128, 1], F32, tag="ss")
            nc.scalar.activation(sq, xc[:], AF.Square, accum_out=ss)
            std = statp.tile([128, 1], F32, tag="std")
            nc.scalar.activation(std, ss, AF.Sqrt, bias=eps_sb[:], scale=1.0 / D)
            rs = statp.tile([128, 1], F32, tag="rs")
            nc.vector.reciprocal(rs, std)
            xh = lnpool.tile([128, D], F32, tag="xh")
            nc.vector.tensor_scalar_mul(xh[:], xc[:], rs)
            c, lo = divmod(bt * 128, CHUNK)
            for kd in range(KD):
                pst = psC.tile([128, 128], F32, tag="tp")
                nc.tensor.transpose(
                    pst[:], xh[:, kd * 128:(kd + 1) * 128], ident[:])
                nc.vector.tensor_copy(
                    xhatT_c[c][:, kd, lo:lo + 128], pst[:])

        # ---------------- Phase 1: experts ----------------
        for e in range(E):
            w2sb = epool.tile([128, MH, E], mm_dt, tag="w2sb")
            b1sb = epool.tile([128, MH], F32, tag="b1sb")
            with nc.allow_non_contiguous_dma(reason="per-expert param loads"):
                nc.sync.dma_start(
                    w2sb[:], w2[e].rearrange("(ko ki) c -> ki ko c", ki=128))
                nc.sync.dma_start(
                    b1sb[:], b1[e].rearrange("(mo mi) -> mi mo", mi=128))

            if affine:
                for kd in range(KD):
                    for c in range(NCH):
                        nc.scalar.activation(
                            x_eT[:, kd, c * CHUNK:(c + 1) * CHUNK],
                            xhatT_c[c][:, kd, :], AF.Identity,
                            bias=betT[:, kd, e:e + 1],
                            scale=gamT[:, kd, e:e + 1])

            def rhs_for(k, c):
                if affine:
                    return x_eT[:, k, c * CHUNK:(c + 1) * CHUNK]
                return xhatT_c[c][:, k, :]

            ps2 = [psB.tile([E, CHUNK], F32, tag="ps2", name=f"ps2_{e}_{c}")
                   for c in range(NCH)]

            for m in range(MH):
                strip = wpool.tile([128, KD, 128], mm_dt, tag="w1s")
                nc.sync.dma_start(
                    strip[:],
                    w1[e, :, m * 128:(m + 1) * 128].rearrange(
                        "(ko ki) m -> ki ko m", ki=128))
                for c in range(NCH):
                    ps1 = psA.tile([128, CHUNK], F32, tag="ps1")
                    for k in range(KD):
                        nc.tensor.matmul(
                            ps1[:],
                            strip[:, k, :],
                            rhs_for(k, c),
                            start=(k == 0), stop=(k == KD - 1))
                    hsb = hpool.tile([128, CHUNK], mm_dt, tag="h")
                    nc.scalar.activation(
                        hsb[:], ps1[:], AF.Relu, bias=b1sb[:, m:m + 1])
                    nc.tensor.matmul(
                        ps2[c][:],
                        w2sb[:, m, :],
                        hsb[:],
                        start=(m == 0), stop=(m == MH - 1))

            # Drain: add b2, transpose back to [B, C], weight by gate, accum.
            for c in range(NCH):
                lsb = spool.tile([E, CHUNK], F32, tag="lsb")
                nc.scalar.activation(
                    lsb[:], ps2[c][:], AF.Identity, bias=b2T4[:E, e:e + 1])
                for sub in range(CHUNK // 128):
                    bt = c * (CHUNK // 128) + sub
                    pst = psC.tile([128, E], F32, tag="tp")
                    nc.tensor.transpose(
                        pst[:], lsb[:, sub * 128:(sub + 1) * 128],
                        ident[:E, :E])
                    tmp = spool.tile([128, E], F32, tag="ltmp")
                    nc.vector.tensor_scalar_mul(
                        tmp[:], pst[:], w_sb[:, bt, e:e + 1])
                    nc.vector.tensor_tensor(
                        acc[:, bt, :], acc[:, bt, :], tmp[:], ALU.add)

        # ---------------- Outputs ----------------
        nc.sync.dma_start(
            logits_o.rearrange("(bo bi) c -> bi bo c", bi=128), acc[:])
        nc.sync.dma_start(
            w_o.rearrange("(bo bi) c -> bi bo c", bi=128), w_sb[:])

    nc.compile()
    return nc


_CACHE = {}


def kernel(**inputs):
    feat = np.ascontiguousarray(inputs["feat"], dtype=np.float32)
    z_cat = np.ascontiguousarray(inputs["z_cat"], dtype=np.float32)
    mu_cat = np.ascontiguousarray(inputs["mu_cat"], dtype=np.float32)
    ln_gamma = np.asarray(inputs["ln_gamma"], dtype=np.float32)
    ln_beta = np.asarray(inputs["ln_beta"], dtype=np.float32)
    W1 = np.ascontiguousarray(inputs["W1"], dtype=np.float32)
    b1 = np.ascontiguousarray(inputs["b1"], dtype=np.float32)
    W2 = np.ascontiguousarray(inputs["W2"], dtype=np.float32)
    b2 = np.ascontiguousarray(inputs["b2"], dtype=np.float32)
    tau = max(1e-6, float(inputs["tau_gate"]))

    affine = not (
        np.all(ln_gamma == 1.0) and np.all(ln_beta == 0.0))

    key = (tau, affine)
    if key not in _CACHE:
        _CACHE[key] = _build(tau, affine)
    nc = _CACHE[key]

    in_maps = []
    for c in range(NCORES):
        rs = slice(c * BS, (c + 1) * BS)
        m = {
            "feat": feat[rs],
            "z": z_cat[rs],
            "mu": mu_cat,
            "w1": W1,
            "b1": b1,
            "w2": W2,
            "b2": b2,
        }
        if affine:
            m["gam"] = ln_gamma
            m["bet"] = ln_beta
        in_maps.append(m)

    res = run_bass_kernel_spmd(nc, in_maps, core_ids=list(range(NCORES)))
    outs = res.results
    logits = np.concatenate([o["logits"] for o in outs], axis=0)
    w = np.concatenate([o["w"] for o in outs], axis=0)
    return logits.astype(np.float32), w.astype(np.float32)

